# revision 40
# baseline (speedup 1.0000x reference)
"""BrahmaAttention (GQA prefill with KV cache) on 8 Trainium2 NeuronCores.

Problem: B=4, S=1024, C=1024 (cache), H=16 q-heads, G=4 kv-heads, D=128.
    q = hs @ wq.T ; k = hs @ wk.T ; v = hs @ wv.T
    rope(q, k) (interleaved pairs, positions C..C+S)
    k_full/v_full = concat(cache, new)           # K = 2048 keys
    out = softmax(q k^T / sqrt(D)) @ v_full @ wo.T
(attention_mask is all-zeros by construction - full attention, no masking.)

Sharding: 4-way data parallel over batch x 2-way tensor parallel over heads.
core (b, hg) handles batch b, q-heads hg*8..hg*8+8, kv-heads hg*2..hg*2+2 and
computes a partial output projection over its 1024 hidden columns; the host
sums the two partials per batch (the TP all-reduce done on host at gather).

Host-side prep folded into the shards:
  - 1/sqrt(D) folded into wq.
  - RoPE even/odd interleave permuted to [evens|odds] via wq/wk row
    permutation and cache_k last-dim permutation, so on-chip RoPE is
    half-tile elementwise ops (partitions 0-63 = even, 64-127 = odd lanes).
  - All tensors pre-cast to bf16 (PSUM accumulation stays fp32) and
    pre-transposed so every DMA lands partition-contiguous (1 descriptor
    per partition) and every matmul operand is naturally [K=128, *].

On-chip layout per core: everything transposed, [D, S]-style:
  qT [128, 8, 1024]  kT_full [128, 2, 2048]  v_full [128, 16, 2, 128]
  scoresT = kT.T @ qT per head, chunk-granular [keys128, s512] PSUM tiles;
  exp on ACT -> probs (bf16); AV accumulates on PE 4 chunks behind scores.
  softmax denominator: chained bf16 adds of the 16 prob chunks on DVE
  + one [128,1]-ones matmul, reciprocal via DVE approx, partition-broadcast
  with a K=1 matmul; normalization fused into the AV PSUM->SBUF copyback.

The wo projection is cut into (m,n) py-tile units of 9 steps each and
software-pipelined ACROSS iterations: each half's units are interleaved
step-by-step into the NEXT half's attention heads (sh0's units fill sh1's
heads, sh1's units fill the next iteration's sh0), so PE bubbles left by
exp (ACT) latency absorb always-ready matmuls. attn/won live in a
cross-iteration ping-pong; the last 4 units' y stores per half are
deferred past the next iteration's input loads so their semaphore waits
don't head-of-line-block the SP DMA queue at the iteration boundary.
wk[g0] and the first NPRE hsT chunks live in persistent prefetch tiles
DMA'd one iteration ahead, so the boundary k-projection starts with zero
load latency. y partials are stored as bf16 and summed in fp32 on host.
"""

import numpy as np
import ml_dtypes

B, S, C, H, G, D = 4, 1024, 1024, 16, 4, 128
HID = H * D
P = 128
NH, NG = 8, 2          # per-core q heads / kv heads
KC = (C + S) // P      # 16 key chunks
KT = 16                # hid contraction tiles
SH = 512               # s-half (PSUM bank free size)
N_CORES = 8
AVLAG = 4              # AV matmul trails the score matmul by this many chunks
NPRE = 4               # hsT chunks prefetched one iteration ahead

_PERM = np.concatenate([np.arange(0, D, 2), np.arange(1, D, 2)])
_BF = ml_dtypes.bfloat16

_BUILT = {}


def _mm(nc, out, lhsT, rhs, **kw):
    nc.tensor.matmul(out, lhsT, rhs, **kw)


def _rope(nc, pool, f32, psum_in, out_ap, cs_cc, cs_pm, mult):
    """out = psum_in*[cos;cos] + swap_halves(psum_in*[sin;-sin]).

    psum_in is the raw projected [128, S] tile with evens on partitions 0-63
    and odds on 64-127; out gets the roped value in the same layout.
    """
    import concourse.mybir as mybir

    bf = _bf()
    a = pool.tile([P, S], bf, tag="ropeA", name="ropeA")
    b = pool.tile([P, S], bf, tag="ropeB", name="ropeB")
    s = pool.tile([P, S], bf, tag="ropeS", name="ropeS")
    nc.vector.tensor_tensor(a[:], psum_in[:], cs_cc[:], mult)
    nc.vector.tensor_tensor(b[:], psum_in[:], cs_pm[:], mult)
    # swap halves on the scalar engine (idle during phase 1)
    nc.scalar.copy(s[0:64, :], b[64:128, :])
    nc.scalar.copy(s[64:128, :], b[0:64, :])
    nc.vector.tensor_tensor(out_ap, a[:], s[:], mybir.AluOpType.add)


def build_bass(unroll=1):
    """Build + compile the per-core Bass program (identical on all cores)."""
    if unroll in _BUILT:
        return _BUILT[unroll]

    import concourse.mybir as mybir
    import concourse.tile as tile
    from concourse import bacc

    f32 = mybir.dt.float32
    f32r = mybir.dt.float32r
    bf = mybir.dt.bfloat16
    mult = mybir.AluOpType.mult
    add = mybir.AluOpType.add
    Exp = mybir.ActivationFunctionType.Exp

    nc = bacc.Bacc("TRN2", target_bir_lowering=False, debug=False)

    hsT_d = nc.dram_tensor("hsT", [P, KT, S], bf, kind="ExternalInput")
    wq_d = nc.dram_tensor("wqT", [NH, P, KT, P], bf, kind="ExternalInput")
    wk_d = nc.dram_tensor("wkT", [NG, P, KT, P], bf, kind="ExternalInput")
    wv_d = nc.dram_tensor("wvT", [P, KT, NG * P], bf, kind="ExternalInput")
    wo_d = nc.dram_tensor("woT", [P, NH, HID], bf, kind="ExternalInput")
    ck_d = nc.dram_tensor("ckT", [NG, P, C], bf, kind="ExternalInput")
    cv_d = nc.dram_tensor("cvP", [P, C // P, NG, P], bf, kind="ExternalInput")
    cc_d = nc.dram_tensor("cs_cc", [P, S], f32, kind="ExternalInput")
    pm_d = nc.dram_tensor("cs_pm", [P, S], f32, kind="ExternalInput")
    y_d = nc.dram_tensor("y", [S, HID], bf, kind="ExternalOutput")

    with tile.TileContext(nc) as tc:
        with (
            tc.tile_pool(name="const", bufs=1) as const,
            tc.tile_pool(name="hold", bufs=1) as hold,
        ):
            ones_f = const.tile([P, P], f32, name="ones_f")
            nc.any.memset(ones_f[:], 1.0)
            ones_bf = const.tile([P, P], bf, name="ones_bf")
            nc.vector.tensor_copy(ones_bf[:], ones_f[:])
            ones1 = const.tile([1, P], f32r, name="ones1")
            nc.vector.tensor_copy(ones1[:], ones_f[0:1, :])
            cs_cc = const.tile([P, S], f32, name="cs_cc")
            cs_pm = const.tile([P, S], f32, name="cs_pm")

            pending = []
            deferred = []
            prefetch = None
            for it in range(unroll):
                pending, prefetch = _emit_iteration(
                    nc, tc, hold, f32, f32r, bf, mult, add, Exp,
                    hsT_d, wq_d, wk_d, wv_d, wo_d, ck_d, cv_d, y_d,
                    ones_bf, ones1, cs_cc, cs_pm, pending, deferred,
                    prefetch, cs_load=(cc_d, pm_d) if it == 0 else None,
                )
            # drain the last iteration's second-half output projection
            if pending:
                with (
                    tc.tile_pool(name="tail_sb", bufs=2) as tail_sb,
                    tc.tile_pool(name="ps_tail", bufs=1, space="PSUM") as ps_t,
                ):
                    ctx = {"ps2": ps_t, "small": tail_sb}
                    from collections import deque
                    fillers = deque(g(ctx) for g in pending)
                    while _pop_filler(fillers):
                        pass
            _flush_stores(nc, y_d, deferred)

    nc.compile()
    _BUILT[unroll] = nc
    return nc


def _pop_filler(fillers):
    """Emit exactly one pending wo-unit step; returns False when drained."""
    while fillers:
        try:
            next(fillers[0])
            return True
        except StopIteration:
            fillers.popleft()
    return False


def _bf():
    import concourse.mybir as mybir
    return mybir.dt.bfloat16


def _wo_unit(nc, ctx, attn_t, won_t, y_d, f32, m, n, defer=None):
    """Generator emitting one (m, n) output-projection unit in 9 steps:
    8 accumulation matmuls + the PSUM->SBUF copy + store. Steps are
    interleaved into attention-head chunk slots so these always-ready
    matmuls absorb PE bubbles left by exp (ACT) latency.

    With defer=(hold_pool, deferred_list) the y store is not emitted here:
    its semaphore wait would sit in the SP DMA queue ahead of the next
    iteration's input loads and head-of-line-block them (measured ~8us PE
    boundary gap + a p-state reset). The ysb tile moves to a persistent
    ring and the store is flushed after the next iteration queues its
    loads."""
    py = None
    for hh in range(NH):
        if hh == 0:
            py = ctx["ps2"].tile([P, SH], f32, tag="py", bufs=2, name="py")
        _mm(nc, py[:], attn_t[:, hh, m * P:(m + 1) * P],
            won_t[:, hh, n * SH:(n + 1) * SH],
            start=(hh == 0), stop=(hh == NH - 1), skip_group_check=True)
        yield
    if defer is None:
        ysb = ctx["small"].tile([P, SH], _bf(), tag="ysb", name="ysb")
        nc.vector.tensor_copy(ysb[:], py[:])
        nc.sync.dma_start(y_d[m * P:(m + 1) * P, n * SH:(n + 1) * SH], ysb[:])
    else:
        hold, deferred = defer
        ysb = hold.tile([P, SH], _bf(), tag="ysbd", bufs=8, name="ysbd")
        nc.vector.tensor_copy(ysb[:], py[:])
        deferred.append((m, n, ysb))
    yield


def _flush_stores(nc, y_d, deferred):
    for m, n, ysb in deferred:
        nc.sync.dma_start(y_d[m * P:(m + 1) * P, n * SH:(n + 1) * SH], ysb[:])
    deferred.clear()


def _emit_iteration(nc, tc, hold, f32, f32r, bf, mult, add, Exp,
                    hsT_d, wq_d, wk_d, wv_d, wo_d, ck_d, cv_d, y_d,
                    ones_bf, ones1, cs_cc, cs_pm, pending, deferred,
                    prefetch, cs_load=None):
    import concourse.tile as tile  # noqa: F401
    from concourse import bass_isa
    from collections import deque

    with tc.tile_pool(name="persist", bufs=1) as persist:
        qT = persist.tile([P, NH, S], bf, name="qT")
        kT = persist.tile([P, NG, C + S], bf, name="kT")
        vF = persist.tile([P, KC, NG, P], bf, name="vF")

        # ---------------- phase 1: projections + rope ----------------
        with (
            tc.tile_pool(name="ph1", bufs=1) as ph1,
            tc.tile_pool(name="wq_pool", bufs=2) as wq_pool,
            tc.tile_pool(name="wk_pool", bufs=1) as wk_pool,
            tc.tile_pool(name="rope", bufs=1) as rope_pool,
            tc.tile_pool(name="ps1", bufs=1, space="PSUM") as ps1,
        ):
            if cs_load is not None:
                # PE warm-up during the initial DMA window (iteration 0
                # only): back-to-back tiny matmuls, consumed by one copy
                # so they survive DCE.
                pw = ps1.tile([P, P], f32, tag="warm", bufs=1, name="pwarm")
                for i in range(150):
                    _mm(nc, pw[:], ones_bf[:], ones_bf[:],
                        start=(i == 0), stop=(i == 149), skip_group_check=True)
                wsink = rope_pool.tile([1, 1], f32, tag="wsink", name="wsink")
                nc.vector.tensor_copy(wsink[:], pw[0:1, 0:1])
            # wk g0 + hsT chunks 0..NPRE live in persistent prefetch tiles
            # whose DMAs were emitted during the PREVIOUS iteration's phase 2
            # (their SBUF WAR cleared at that iteration's phase-1 end), so
            # the boundary k-projection starts without waiting on loads.
            if prefetch is None:
                wk_pre = hold.tile([P, KT, P], bf, tag="wkp", bufs=1,
                                   name="wkp")
                hs_pre = hold.tile([P, NPRE, S], bf, tag="hsp", bufs=1,
                                   name="hsp")
                nc.sync.dma_start(wk_pre[:], wk_d[0])
                nc.sync.dma_start(hs_pre[:], hsT_d[:, 0:NPRE, :])
            else:
                wk_pre, hs_pre = prefetch
            # DMA emission order = queue order: k-proj consumes the
            # prefetched chunks first, so the remaining hsT chunks go ahead
            # of bulk loads (cache, wv) only needed later in the phase.
            hsT = ph1.tile([P, KT - NPRE, S], bf, name="hsT_sb")
            wk1 = wk_pool.tile([P, KT, P], bf, tag="wk", name="wk_sb")
            nc.sync.dma_start(wk1[:], wk_d[1])
            if cs_load is not None:
                nc.sync.dma_start(cs_cc[:], cs_load[0][:])
                nc.sync.dma_start(cs_pm[:], cs_load[1][:])
            for i in range(0, (KT - NPRE) // 2):
                nc.sync.dma_start(
                    hsT[:, 2 * i:2 * i + 2, :],
                    hsT_d[:, NPRE + 2 * i:NPRE + 2 * i + 2, :])

            def hs(k, sl):
                return (hs_pre[:, k, sl] if k < NPRE
                        else hsT[:, k - NPRE, sl])
            # q weights for the first heads ahead of the bulk cache loads
            wqs = {}
            for h in range(2):
                wq = wq_pool.tile([P, KT, P], bf, tag="wq", name="wq_sb")
                nc.sync.dma_start(wq[:], wq_d[h])
                wqs[h] = wq
            for g in range(NG):
                nc.sync.dma_start(kT[:, g, 0:C], ck_d[g])
            nc.sync.dma_start(vF[:, 0:C // P, :, :], cv_d[:])
            wv = ph1.tile([P, KT, NG * P], bf, name="wv_sb")
            nc.sync.dma_start(wv[:], wv_d[:])
            # attn/won ping-pong across iterations so the pending wo units of
            # iteration i-1 can read their buffers while iteration i loads.
            attn = hold.tile([P, NH, S], bf, tag="attn", bufs=2, name="attn_sb")
            won = hold.tile([P, NH, HID], bf, tag="won", bufs=2, name="won")
            nc.sync.dma_start(won[:], wo_d[:])
            # previous iteration's deferred y stores go out here, BEHIND this
            # iteration's input loads in the SP queue
            _flush_stores(nc, y_d, deferred)

            # k projection + rope (new keys go to kT[:, g, C:])
            for g in range(NG):
                wk = wk_pre if g == 0 else wk1
                pk = ps1.tile([P, S], f32, tag="pqk", bufs=2, name="pk")
                for k in range(KT):
                    for n in range(2):
                        _mm(
                            nc,
                            pk[:, n * SH:(n + 1) * SH], wk[:, k, :],
                            hs(k, slice(n * SH, (n + 1) * SH)),
                            start=(k == 0), stop=(k == KT - 1),
                        )
                _rope(nc, rope_pool, f32, pk, kT[:, g, C:C + S], cs_cc, cs_pm, mult)

            # q projection + rope
            for h in range(NH):
                if h in wqs:
                    wq = wqs[h]
                else:
                    wq = wq_pool.tile([P, KT, P], bf, tag="wq", name="wq_sb")
                    nc.sync.dma_start(wq[:], wq_d[h])
                pq = ps1.tile([P, S], f32, tag="pqk", bufs=2, name="pq")
                for k in range(KT):
                    for n in range(2):
                        _mm(
                            nc,
                            pq[:, n * SH:(n + 1) * SH], wq[:, k, :],
                            hs(k, slice(n * SH, (n + 1) * SH)),
                            start=(k == 0), stop=(k == KT - 1),
                        )
                _rope(nc, rope_pool, f32, pq, qT[:, h, :], cs_cc, cs_pm, mult)

            # v projection (natural layout: tokens on partitions)
            for mv in range(S // P):
                pv = ps1.tile([P, NG * P], f32, tag="pv", bufs=2, name="pv")
                for k in range(KT):
                    _mm(
                        nc,
                        pv[:], hs(k, slice(mv * P, (mv + 1) * P)), wv[:, k, :],
                        start=(k == 0), stop=(k == KT - 1),
                    )
                nc.vector.tensor_copy(vF[:, C // P + mv, :, :], pv[:])

        # ---------- phase 2: attention + output projection ----------
        with (
            tc.tile_pool(name="probs", bufs=1) as probs_pool,
            tc.tile_pool(name="small", bufs=2) as small_pool,
            tc.tile_pool(name="ps2", bufs=1, space="PSUM") as ps2,
        ):
            # next iteration's prefetch: the targets' WAR (this iteration's
            # phase-1 reads) just cleared, so these DMAs run immediately and
            # the data is resident long before the iteration boundary.
            wk_pre_n = hold.tile([P, KT, P], bf, tag="wkp", bufs=1,
                                 name="wkp")
            hs_pre_n = hold.tile([P, NPRE, S], bf, tag="hsp", bufs=1,
                                 name="hsp")
            nc.sync.dma_start(wk_pre_n[:], wk_d[0])
            nc.sync.dma_start(hs_pre_n[:], hsT_d[:, 0:NPRE, :])

            ctx = {"ps2": ps2, "small": small_pool}
            fillers = deque(g(ctx) for g in pending)

            def attn_head(sh, h):
                ssl = slice(sh * SH, (sh + 1) * SH)
                g = h // (NH // NG)
                pt = probs_pool.tile([P, KC, SH], bf, tag="probs", bufs=2,
                                     name="pt")
                pav = ps2.tile([P, SH], f32, tag="av", bufs=3, name="pav")
                acc = [None]

                def s_emit(c):
                    ps = ps2.tile([P, SH], f32, tag="score", bufs=2,
                                  name="ps")
                    _mm(nc, ps[:], kT[:, g, c * P:(c + 1) * P],
                        qT[:, h, ssl], start=True, stop=True,
                        skip_group_check=True)
                    nc.scalar.activation(pt[:, c, :], ps[:], Exp)
                    # chained bf16 denominator accumulation on DVE
                    if c >= 1:
                        a = small_pool.tile([P, SH], bf, tag="acc", bufs=2,
                                            name="acc")
                        lhs = pt[:, 0, :] if c == 1 else acc[0][:]
                        nc.vector.tensor_tensor(a[:], lhs, pt[:, c, :], add)
                        acc[0] = a

                def av_emit(c):
                    _mm(nc, pav[:], vF[:, c, g, :], pt[:, c, :],
                        start=(c == 0), stop=(c == KC - 1),
                        skip_group_check=True)

                for c in range(KC + AVLAG):
                    if c < KC:
                        s_emit(c)
                    if c >= AVLAG:
                        av_emit(c - AVLAG)
                    if c >= 2:
                        # one wo-unit step per chunk slot (18 slots/head x
                        # 8 heads exactly drains 16 9-step units per half)
                        _pop_filler(fillers)

                # cross-partition sum of the accumulated probs -> denominator
                # (tiny [128,1]-ones matmul), reciprocal on DVE, partition
                # broadcast with a K=1 matmul. (A GpSimd partition_all_reduce
                # is ~10x slower on real hardware than the cost model says -
                # measured u5 per-call regression - so this stays on PE.)
                pd = ps2.tile([P, SH], f32, tag="dn", bufs=1, name="pd")
                _mm(nc, pd[0:1, :], ones_bf[:, 0:1], acc[0][:],
                    start=True, stop=True, skip_group_check=True)
                denr = small_pool.tile([1, SH], f32, tag="denr", name="denr")
                nc.vector.reciprocal_approx_fast(out=denr[:], in_=pd[0:1, :])
                denr_r = small_pool.tile([1, SH], f32r, tag="denr_r",
                                         name="denr_r")
                nc.vector.tensor_copy(denr_r[:], denr[:])
                pb = ps2.tile([P, SH], f32, tag="dn", bufs=1, name="pb")
                _mm(nc, pb[:], ones1[:], denr_r[:], start=True, stop=True,
                    skip_group_check=True)
                rbc = small_pool.tile([P, SH], bf, tag="rbc", name="rbc")
                nc.vector.tensor_copy(rbc[:], pb[:])
                # normalized attention output (transposed), fused copyback
                nc.vector.tensor_tensor(attn[:, h, ssl], pav[:], rbc[:], mult)

            for h in range(NH):
                attn_head(0, h)
            while _pop_filler(fillers):    # safety drain (no-op when exact)
                pass
            units0 = [(m, n) for m in range(4) for n in range(HID // SH)]
            fillers = deque(
                _wo_unit(nc, ctx, attn, won, y_d, f32, m, n,
                         defer=(hold, deferred) if i >= len(units0) - 4
                         else None)
                for i, (m, n) in enumerate(units0))
            for h in range(NH):
                attn_head(1, h)
            while _pop_filler(fillers):
                pass
            # second half's output projection: handed to the NEXT iteration
            # (or the program tail) so its matmuls fill that phase's bubbles
            units1 = [(m + 4, n) for m in range(4) for n in range(HID // SH)]
            nu1 = len(units1)
            return [
                (lambda c, _m=m, _n=n, _d=(i >= nu1 - 4):
                 _wo_unit(nc, c, attn, won, y_d, f32, _m, _n,
                          defer=(hold, deferred) if _d else None))
                for i, (m, n) in enumerate(units1)
            ], (wk_pre_n, hs_pre_n)


def prep_inputs(hidden_states, freqs_cos, freqs_sin, cache_k, cache_v,
                wq, wk, wv, wo):
    """Shard + pre-transpose + bf16-cast the full inputs into 8 per-core
    input maps. All DMA'd tensors are laid out partition-major so every
    transfer is contiguous within a partition."""
    f = np.float32
    scale = np.float32(1.0 / np.sqrt(D))
    wq_p = (wq.astype(f).reshape(H, D, HID)[:, _PERM, :] * scale)
    wk_p = wk.astype(f).reshape(G, D, HID)[:, _PERM, :]
    wv_r = wv.astype(f).reshape(G, D, HID)

    cc = freqs_cos.astype(f).T          # [64, S]
    ss = freqs_sin.astype(f).T
    cs_cc = np.ascontiguousarray(np.concatenate([cc, cc], axis=0))
    cs_pm = np.ascontiguousarray(np.concatenate([ss, -ss], axis=0))

    in_maps = []
    for b in range(B):
        hsT = np.ascontiguousarray(
            hidden_states[b].astype(f).T.reshape(KT, P, S)
            .transpose(1, 0, 2).astype(_BF))            # [P, KT, S]
        for hg in range(2):
            hs_q = slice(hg * NH, (hg + 1) * NH)
            hs_kv = slice(hg * NG, (hg + 1) * NG)
            wqT = wq_p[hs_q].reshape(NH * D, HID).T          # [HID, 1024]
            wqT_t = np.ascontiguousarray(
                wqT.reshape(KT, P, NH, P).transpose(2, 1, 0, 3).astype(_BF))
            wkT = wk_p[hs_kv].reshape(NG * D, HID).T         # [HID, 256]
            wkT_t = np.ascontiguousarray(
                wkT.reshape(KT, P, NG, P).transpose(2, 1, 0, 3).astype(_BF))
            wvT = wv_r[hs_kv].reshape(NG * D, HID).T         # [HID, 256]
            wvT_t = np.ascontiguousarray(
                wvT.reshape(KT, P, NG * P).transpose(1, 0, 2).astype(_BF))
            woT = np.ascontiguousarray(
                wo.astype(f)[:, hg * NH * D:(hg + 1) * NH * D].T
                .reshape(NH, P, HID).transpose(1, 0, 2).astype(_BF))
            ckT = np.ascontiguousarray(
                cache_k[b].astype(f)[:, hs_kv][:, :, _PERM]
                .transpose(1, 2, 0).astype(_BF))
            cvP = np.ascontiguousarray(
                cache_v[b].astype(f)[:, hs_kv]
                .reshape(C // P, P, NG, P).transpose(1, 0, 2, 3).astype(_BF))
            in_maps.append({
                "hsT": hsT, "wqT": wqT_t, "wkT": wkT_t, "wvT": wvT_t,
                "woT": woT, "ckT": ckT, "cvP": cvP,
                "cs_cc": cs_cc, "cs_pm": cs_pm,
            })
    return in_maps


def gather_output(results):
    """Sum the 2 TP partials per batch -> full [B, S, HID] output."""
    out = np.empty((B, S, HID), np.float32)
    for b in range(B):
        out[b] = (results[2 * b]["y"].astype(np.float32)
                  + results[2 * b + 1]["y"].astype(np.float32))
    return out


def kernel(hidden_states, freqs_cos, freqs_sin, attention_mask,
           cache_k, cache_v, wq, wk, wv, wo):
    # attention_mask is all-zeros by construction (see spec) - unused.
    from concourse.bass_utils import run_bass_kernel_spmd

    nc = build_bass(unroll=1)
    in_maps = prep_inputs(
        np.asarray(hidden_states), np.asarray(freqs_cos), np.asarray(freqs_sin),
        np.asarray(cache_k), np.asarray(cache_v),
        np.asarray(wq), np.asarray(wk), np.asarray(wv), np.asarray(wo))
    res = run_bass_kernel_spmd(nc, in_maps, core_ids=list(range(N_CORES)))
    return gather_output(res.results)


# revision 43
# speedup vs baseline: 1.1468x; 1.1468x over previous
"""BrahmaAttention (GQA prefill with KV cache) on 8 Trainium2 NeuronCores.

Problem: B=4, S=1024, C=1024 (cache), H=16 q-heads, G=4 kv-heads, D=128.
    q = hs @ wq.T ; k = hs @ wk.T ; v = hs @ wv.T
    rope(q, k) (interleaved pairs, positions C..C+S)
    k_full/v_full = concat(cache, new)           # K = 2048 keys
    out = softmax(q k^T / sqrt(D)) @ v_full @ wo.T
(attention_mask is all-zeros by construction - full attention, no masking.)

Sharding: 4-way data parallel over batch x 2-way tensor parallel over heads.
core (b, hg) handles batch b, q-heads hg*8..hg*8+8, kv-heads hg*2..hg*2+2 and
computes a partial output projection over its 1024 hidden columns; the host
sums the two partials per batch (the TP all-reduce done on host at gather).

Host-side prep folded into the shards:
  - 1/sqrt(D) folded into wq.
  - RoPE even/odd interleave permuted to [evens|odds] via wq/wk row
    permutation and cache_k last-dim permutation, so on-chip RoPE is
    half-tile elementwise ops (partitions 0-63 = even, 64-127 = odd lanes).
  - All tensors pre-cast to bf16 (PSUM accumulation stays fp32) and
    pre-transposed so every DMA lands partition-contiguous (1 descriptor
    per partition) and every matmul operand is naturally [K=128, *].

On-chip layout per core: everything transposed, [D, S]-style:
  qT [128, 8, 1024]  kT_full [128, 2, 2048]  v_full [128, 16, 2, 128]
  scoresT = kT.T @ qT per head, chunk-granular [keys128, s512] PSUM tiles;
  exp on ACT -> probs (bf16); AV accumulates on PE 4 chunks behind scores.
  softmax denominator: chained bf16 adds of the 16 prob chunks on DVE
  + one [128,1]-ones matmul, reciprocal via DVE approx, partition-broadcast
  with a K=1 matmul; normalization fused into the AV PSUM->SBUF copyback.

The wo projection is cut into (m,n) py-tile units of 9 steps each and
software-pipelined ACROSS iterations: each half's units are interleaved
step-by-step into the NEXT half's attention heads (sh0's units fill sh1's
heads, sh1's units fill the next iteration's sh0), so PE bubbles left by
exp (ACT) latency absorb always-ready matmuls. attn/won live in a
cross-iteration ping-pong; the last 4 units' y stores per half are
deferred past the next iteration's input loads so their semaphore waits
don't head-of-line-block the SP DMA queue at the iteration boundary.
wk[g0] and the first NPRE hsT chunks live in persistent prefetch tiles
DMA'd one iteration ahead, so the boundary k-projection starts with zero
load latency. y partials are stored as bf16 and summed in fp32 on host.
"""

import numpy as np
import ml_dtypes

B, S, C, H, G, D = 4, 1024, 1024, 16, 4, 128
HID = H * D
P = 128
NH, NG = 8, 2          # per-core q heads / kv heads
KC = (C + S) // P      # 16 key chunks
KT = 16                # hid contraction tiles
SH = 512               # s-half (PSUM bank free size)
N_CORES = 8
AVLAG = 4              # AV matmul trails the score matmul by this many chunks
NPRE = 4               # hsT chunks prefetched one iteration ahead

_PERM = np.concatenate([np.arange(0, D, 2), np.arange(1, D, 2)])
_BF = ml_dtypes.bfloat16

_BUILT = {}


def _mm(nc, out, lhsT, rhs, **kw):
    nc.tensor.matmul(out, lhsT, rhs, **kw)


def _rope(nc, pool, f32, psum_in, out_ap, cs_cc, cs_pm, mult):
    """out = psum_in*[cos;cos] + swap_halves(psum_in*[sin;-sin]).

    psum_in is the raw projected [128, S] tile with evens on partitions 0-63
    and odds on 64-127; out gets the roped value in the same layout.
    """
    import concourse.mybir as mybir

    bf = _bf()
    a = pool.tile([P, S], bf, tag="ropeA", name="ropeA")
    b = pool.tile([P, S], bf, tag="ropeB", name="ropeB")
    s = pool.tile([P, S], bf, tag="ropeS", name="ropeS")
    nc.vector.tensor_tensor(a[:], psum_in[:], cs_cc[:], mult)
    nc.vector.tensor_tensor(b[:], psum_in[:], cs_pm[:], mult)
    # swap halves on the scalar engine (idle during phase 1)
    nc.scalar.copy(s[0:64, :], b[64:128, :])
    nc.scalar.copy(s[64:128, :], b[0:64, :])
    nc.vector.tensor_tensor(out_ap, a[:], s[:], mybir.AluOpType.add)


def build_bass(unroll=1):
    """Build + compile the per-core Bass program (identical on all cores)."""
    if unroll in _BUILT:
        return _BUILT[unroll]

    import concourse.mybir as mybir
    import concourse.tile as tile
    from concourse import bacc

    f32 = mybir.dt.float32
    f32r = mybir.dt.float32r
    bf = mybir.dt.bfloat16
    mult = mybir.AluOpType.mult
    add = mybir.AluOpType.add
    Exp = mybir.ActivationFunctionType.Exp

    nc = bacc.Bacc("TRN2", target_bir_lowering=False, debug=False)

    hsT_d = nc.dram_tensor("hsT", [P, KT, S], bf, kind="ExternalInput")
    wq_d = nc.dram_tensor("wqT", [NH, P, KT, P], bf, kind="ExternalInput")
    wk_d = nc.dram_tensor("wkT", [NG, P, KT, P], bf, kind="ExternalInput")
    wv_d = nc.dram_tensor("wvT", [P, KT, NG * P], bf, kind="ExternalInput")
    wo_d = nc.dram_tensor("woT", [P, NH, HID], bf, kind="ExternalInput")
    ck_d = nc.dram_tensor("ckT", [NG, P, C], bf, kind="ExternalInput")
    cv_d = nc.dram_tensor("cvP", [P, C // P, NG, P], bf, kind="ExternalInput")
    cc_d = nc.dram_tensor("cs_cc", [P, S], f32, kind="ExternalInput")
    pm_d = nc.dram_tensor("cs_pm", [P, S], f32, kind="ExternalInput")
    y_d = nc.dram_tensor("y", [S, HID], bf, kind="ExternalOutput")

    with tile.TileContext(nc) as tc:
        with (
            tc.tile_pool(name="const", bufs=1) as const,
            tc.tile_pool(name="hold", bufs=1) as hold,
        ):
            ones_f = const.tile([P, P], f32, name="ones_f")
            nc.any.memset(ones_f[:], 1.0)
            ones_bf = const.tile([P, P], bf, name="ones_bf")
            nc.vector.tensor_copy(ones_bf[:], ones_f[:])
            ones1 = const.tile([1, P], f32r, name="ones1")
            nc.vector.tensor_copy(ones1[:], ones_f[0:1, :])
            cs_cc = const.tile([P, S], f32, name="cs_cc")
            cs_pm = const.tile([P, S], f32, name="cs_pm")

            pending = []
            deferred = []
            prefetch = None
            for it in range(unroll):
                pending, prefetch = _emit_iteration(
                    nc, tc, hold, f32, f32r, bf, mult, add, Exp,
                    hsT_d, wq_d, wk_d, wv_d, wo_d, ck_d, cv_d, y_d,
                    ones_bf, ones1, cs_cc, cs_pm, pending, deferred,
                    prefetch, cs_load=(cc_d, pm_d) if it == 0 else None,
                )
            # drain the last iteration's second-half output projection
            if pending:
                with (
                    tc.tile_pool(name="tail_sb", bufs=2) as tail_sb,
                    tc.tile_pool(name="ps_tail", bufs=1, space="PSUM") as ps_t,
                ):
                    ctx = {"ps2": ps_t, "small": tail_sb}
                    from collections import deque
                    fillers = deque(g(ctx) for g in pending)
                    while _pop_filler(fillers):
                        pass
            _flush_stores(nc, y_d, deferred)

    nc.compile()
    _BUILT[unroll] = nc
    return nc


def _pop_filler(fillers):
    """Emit exactly one pending wo-unit step; returns False when drained."""
    while fillers:
        try:
            next(fillers[0])
            return True
        except StopIteration:
            fillers.popleft()
    return False


def _bf():
    import concourse.mybir as mybir
    return mybir.dt.bfloat16


def _wo_unit(nc, ctx, attn_t, won_t, y_d, f32, m, n, defer=None):
    """Generator emitting one (m, n) output-projection unit in 9 steps:
    8 accumulation matmuls + the PSUM->SBUF copy + store. Steps are
    interleaved into attention-head chunk slots so these always-ready
    matmuls absorb PE bubbles left by exp (ACT) latency.

    With defer=(hold_pool, deferred_list) the y store is not emitted here:
    its semaphore wait would sit in the SP DMA queue ahead of the next
    iteration's input loads and head-of-line-block them (measured ~8us PE
    boundary gap + a p-state reset). The ysb tile moves to a persistent
    ring and the store is flushed after the next iteration queues its
    loads."""
    py = None
    for hh in range(NH):
        if hh == 0:
            py = ctx["ps2"].tile([P, SH], f32, tag="py", bufs=2, name="py")
        _mm(nc, py[:], attn_t[:, hh, m * P:(m + 1) * P],
            won_t[:, hh, n * SH:(n + 1) * SH],
            start=(hh == 0), stop=(hh == NH - 1), skip_group_check=True)
        yield
    if defer is None:
        ysb = ctx["small"].tile([P, SH], _bf(), tag="ysb", name="ysb")
        nc.vector.tensor_copy(ysb[:], py[:])
        nc.sync.dma_start(y_d[m * P:(m + 1) * P, n * SH:(n + 1) * SH], ysb[:])
    else:
        hold, deferred = defer
        ysb = hold.tile([P, SH], _bf(), tag="ysbd", bufs=8, name="ysbd")
        nc.vector.tensor_copy(ysb[:], py[:])
        deferred.append((m, n, ysb))
    yield


def _flush_stores(nc, y_d, deferred):
    for m, n, ysb in deferred:
        nc.sync.dma_start(y_d[m * P:(m + 1) * P, n * SH:(n + 1) * SH], ysb[:])
    deferred.clear()


def _emit_iteration(nc, tc, hold, f32, f32r, bf, mult, add, Exp,
                    hsT_d, wq_d, wk_d, wv_d, wo_d, ck_d, cv_d, y_d,
                    ones_bf, ones1, cs_cc, cs_pm, pending, deferred,
                    prefetch, cs_load=None):
    import concourse.tile as tile  # noqa: F401
    from concourse import bass_isa
    from collections import deque

    with tc.tile_pool(name="persist", bufs=1) as persist:
        qT = persist.tile([P, NH, S], bf, name="qT")
        kT = persist.tile([P, NG, C + S], bf, name="kT")
        vF = persist.tile([P, KC, NG, P], bf, name="vF")

        # ---------------- phase 1: projections + rope ----------------
        with (
            tc.tile_pool(name="ph1", bufs=1) as ph1,
            tc.tile_pool(name="wq_pool", bufs=2) as wq_pool,
            tc.tile_pool(name="wk_pool", bufs=1) as wk_pool,
            tc.tile_pool(name="rope", bufs=1) as rope_pool,
            tc.tile_pool(name="ps1", bufs=1, space="PSUM") as ps1,
        ):
            if cs_load is not None:
                # PE warm-up during the initial DMA window (iteration 0
                # only): back-to-back tiny matmuls, consumed by one copy
                # so they survive DCE.
                pw = ps1.tile([P, P], f32, tag="warm", bufs=1, name="pwarm")
                for i in range(150):
                    _mm(nc, pw[:], ones_bf[:], ones_bf[:],
                        start=(i == 0), stop=(i == 149), skip_group_check=True)
                wsink = rope_pool.tile([1, 1], f32, tag="wsink", name="wsink")
                nc.vector.tensor_copy(wsink[:], pw[0:1, 0:1])
            # wk g0 + hsT chunks 0..NPRE live in persistent prefetch tiles
            # whose DMAs were emitted during the PREVIOUS iteration's phase 2
            # (their SBUF WAR cleared at that iteration's phase-1 end), so
            # the boundary k-projection starts without waiting on loads.
            if prefetch is None:
                wk_pre = hold.tile([P, KT, P], bf, tag="wkp", bufs=1,
                                   name="wkp")
                hs_pre = hold.tile([P, NPRE, S], bf, tag="hsp", bufs=1,
                                   name="hsp")
                nc.sync.dma_start(wk_pre[:], wk_d[0])
                nc.sync.dma_start(hs_pre[:], hsT_d[:, 0:NPRE, :])
            else:
                wk_pre, hs_pre = prefetch
            # DMA emission order = queue order: k-proj consumes the
            # prefetched chunks first, so the remaining hsT chunks go ahead
            # of bulk loads (cache, wv) only needed later in the phase.
            hsT = ph1.tile([P, KT - NPRE, S], bf, name="hsT_sb")
            wk1 = wk_pool.tile([P, KT, P], bf, tag="wk", name="wk_sb")
            nc.sync.dma_start(wk1[:], wk_d[1])
            if cs_load is not None:
                nc.sync.dma_start(cs_cc[:], cs_load[0][:])
                nc.sync.dma_start(cs_pm[:], cs_load[1][:])
            for i in range(0, (KT - NPRE) // 2):
                nc.sync.dma_start(
                    hsT[:, 2 * i:2 * i + 2, :],
                    hsT_d[:, NPRE + 2 * i:NPRE + 2 * i + 2, :])

            def hs(k, sl):
                return (hs_pre[:, k, sl] if k < NPRE
                        else hsT[:, k - NPRE, sl])
            # q weights for the first heads ahead of the bulk cache loads
            wqs = {}
            for h in range(2):
                wq = wq_pool.tile([P, KT, P], bf, tag="wq", name="wq_sb")
                nc.sync.dma_start(wq[:], wq_d[h])
                wqs[h] = wq
            for g in range(NG):
                nc.sync.dma_start(kT[:, g, 0:C], ck_d[g])
            nc.sync.dma_start(vF[:, 0:C // P, :, :], cv_d[:])
            wv = ph1.tile([P, KT, NG * P], bf, name="wv_sb")
            nc.sync.dma_start(wv[:], wv_d[:])
            # attn/won ping-pong across iterations so the pending wo units of
            # iteration i-1 can read their buffers while iteration i loads.
            attn = hold.tile([P, NH, S], bf, tag="attn", bufs=2, name="attn_sb")
            won = hold.tile([P, NH, HID], bf, tag="won", bufs=2, name="won")
            nc.sync.dma_start(won[:], wo_d[:])
            # previous iteration's deferred y stores go out here, BEHIND this
            # iteration's input loads in the SP queue
            _flush_stores(nc, y_d, deferred)

            # k projection + rope (new keys go to kT[:, g, C:])
            for g in range(NG):
                wk = wk_pre if g == 0 else wk1
                pk = ps1.tile([P, S], f32, tag="pqk", bufs=2, name="pk")
                for k in range(KT):
                    for n in range(2):
                        _mm(
                            nc,
                            pk[:, n * SH:(n + 1) * SH], wk[:, k, :],
                            hs(k, slice(n * SH, (n + 1) * SH)),
                            start=(k == 0), stop=(k == KT - 1),
                        )
                _rope(nc, rope_pool, f32, pk, kT[:, g, C:C + S], cs_cc, cs_pm, mult)

            # q projection + rope
            for h in range(NH):
                if h in wqs:
                    wq = wqs[h]
                else:
                    wq = wq_pool.tile([P, KT, P], bf, tag="wq", name="wq_sb")
                    nc.sync.dma_start(wq[:], wq_d[h])
                pq = ps1.tile([P, S], f32, tag="pqk", bufs=2, name="pq")
                for k in range(KT):
                    for n in range(2):
                        _mm(
                            nc,
                            pq[:, n * SH:(n + 1) * SH], wq[:, k, :],
                            hs(k, slice(n * SH, (n + 1) * SH)),
                            start=(k == 0), stop=(k == KT - 1),
                        )
                _rope(nc, rope_pool, f32, pq, qT[:, h, :], cs_cc, cs_pm, mult)

            # v projection (natural layout: tokens on partitions)
            for mv in range(S // P):
                pv = ps1.tile([P, NG * P], f32, tag="pv", bufs=2, name="pv")
                for k in range(KT):
                    _mm(
                        nc,
                        pv[:], hs(k, slice(mv * P, (mv + 1) * P)), wv[:, k, :],
                        start=(k == 0), stop=(k == KT - 1),
                    )
                nc.vector.tensor_copy(vF[:, C // P + mv, :, :], pv[:])

        # ---------- phase 2: attention + output projection ----------
        with (
            tc.tile_pool(name="probs", bufs=1) as probs_pool,
            tc.tile_pool(name="small", bufs=2) as small_pool,
            tc.tile_pool(name="ps2", bufs=1, space="PSUM") as ps2,
        ):
            # next iteration's prefetch: the targets' WAR (this iteration's
            # phase-1 reads) just cleared, so these DMAs run immediately and
            # the data is resident long before the iteration boundary.
            wk_pre_n = hold.tile([P, KT, P], bf, tag="wkp", bufs=1,
                                 name="wkp")
            hs_pre_n = hold.tile([P, NPRE, S], bf, tag="hsp", bufs=1,
                                 name="hsp")
            nc.sync.dma_start(wk_pre_n[:], wk_d[0])
            nc.sync.dma_start(hs_pre_n[:], hsT_d[:, 0:NPRE, :])

            ctx = {"ps2": ps2, "small": small_pool}
            fillers = deque(g(ctx) for g in pending)

            def attn_head(sh, h):
                ssl = slice(sh * SH, (sh + 1) * SH)
                g = h // (NH // NG)
                pt = probs_pool.tile([P, KC, SH], bf, tag="probs", bufs=2,
                                     name="pt")
                pav = ps2.tile([P, SH], f32, tag="av", bufs=3, name="pav")
                acc = [None]

                def s_emit(c):
                    ps = ps2.tile([P, SH], f32, tag="score", bufs=2,
                                  name="ps")
                    _mm(nc, ps[:], kT[:, g, c * P:(c + 1) * P],
                        qT[:, h, ssl], start=True, stop=True,
                        skip_group_check=True)
                    nc.scalar.activation(pt[:, c, :], ps[:], Exp)
                    # chained bf16 denominator accumulation on DVE
                    if c >= 1:
                        a = small_pool.tile([P, SH], bf, tag="acc", bufs=2,
                                            name="acc")
                        lhs = pt[:, 0, :] if c == 1 else acc[0][:]
                        nc.vector.tensor_tensor(a[:], lhs, pt[:, c, :], add)
                        acc[0] = a

                def av_emit(c):
                    _mm(nc, pav[:], vF[:, c, g, :], pt[:, c, :],
                        start=(c == 0), stop=(c == KC - 1),
                        skip_group_check=True)

                for c in range(KC + AVLAG):
                    if c < KC:
                        s_emit(c)
                    if c >= AVLAG:
                        av_emit(c - AVLAG)
                    if c >= 2:
                        # one wo-unit step per chunk slot (18 slots/head x
                        # 8 heads exactly drains 16 9-step units per half)
                        _pop_filler(fillers)

                # cross-partition sum of the accumulated probs -> denominator
                # (tiny [128,1]-ones matmul), reciprocal on DVE, partition
                # broadcast with a K=1 matmul. (A GpSimd partition_all_reduce
                # is ~10x slower on real hardware than the cost model says -
                # measured u5 per-call regression - so this stays on PE.)
                pd = ps2.tile([P, SH], f32, tag="dn", bufs=1, name="pd")
                _mm(nc, pd[0:1, :], ones_bf[:, 0:1], acc[0][:],
                    start=True, stop=True, skip_group_check=True)
                denr = small_pool.tile([1, SH], f32, tag="denr", name="denr")
                nc.vector.reciprocal_approx_fast(out=denr[:], in_=pd[0:1, :])
                denr_r = small_pool.tile([1, SH], f32r, tag="denr_r",
                                         name="denr_r")
                nc.vector.tensor_copy(denr_r[:], denr[:])
                pb = ps2.tile([P, SH], f32, tag="dn", bufs=1, name="pb")
                _mm(nc, pb[:], ones1[:], denr_r[:], start=True, stop=True,
                    skip_group_check=True)
                rbc = small_pool.tile([P, SH], bf, tag="rbc", name="rbc")
                nc.vector.tensor_copy(rbc[:], pb[:])
                # normalized attention output (transposed), fused copyback
                nc.vector.tensor_tensor(attn[:, h, ssl], pav[:], rbc[:], mult)

            for h in range(NH):
                attn_head(0, h)
            while _pop_filler(fillers):    # safety drain (no-op when exact)
                pass
            units0 = [(m, n) for m in range(4) for n in range(HID // SH)]
            fillers = deque(
                _wo_unit(nc, ctx, attn, won, y_d, f32, m, n,
                         defer=(hold, deferred) if i >= len(units0) - 4
                         else None)
                for i, (m, n) in enumerate(units0))
            for h in range(NH):
                attn_head(1, h)
            while _pop_filler(fillers):
                pass
            # second half's output projection: handed to the NEXT iteration
            # (or the program tail) so its matmuls fill that phase's bubbles
            units1 = [(m + 4, n) for m in range(4) for n in range(HID // SH)]
            nu1 = len(units1)
            return [
                (lambda c, _m=m, _n=n, _d=(i >= nu1 - 4):
                 _wo_unit(nc, c, attn, won, y_d, f32, _m, _n,
                          defer=(hold, deferred) if _d else None))
                for i, (m, n) in enumerate(units1)
            ], (wk_pre_n, hs_pre_n)


def prep_inputs(hidden_states, freqs_cos, freqs_sin, cache_k, cache_v,
                wq, wk, wv, wo):
    """Shard + pre-transpose + bf16-cast the full inputs into 8 per-core
    input maps. All DMA'd tensors are laid out partition-major so every
    transfer is contiguous within a partition."""
    f = np.float32
    scale = np.float32(1.0 / np.sqrt(D))
    wq_p = (wq.astype(f).reshape(H, D, HID)[:, _PERM, :] * scale)
    wk_p = wk.astype(f).reshape(G, D, HID)[:, _PERM, :]
    wv_r = wv.astype(f).reshape(G, D, HID)

    cc = freqs_cos.astype(f).T          # [64, S]
    ss = freqs_sin.astype(f).T
    cs_cc = np.ascontiguousarray(np.concatenate([cc, cc], axis=0))
    cs_pm = np.ascontiguousarray(np.concatenate([ss, -ss], axis=0))

    in_maps = []
    for b in range(B):
        hsT = np.ascontiguousarray(
            hidden_states[b].astype(f).T.reshape(KT, P, S)
            .transpose(1, 0, 2).astype(_BF))            # [P, KT, S]
        for hg in range(2):
            hs_q = slice(hg * NH, (hg + 1) * NH)
            hs_kv = slice(hg * NG, (hg + 1) * NG)
            wqT = wq_p[hs_q].reshape(NH * D, HID).T          # [HID, 1024]
            wqT_t = np.ascontiguousarray(
                wqT.reshape(KT, P, NH, P).transpose(2, 1, 0, 3).astype(_BF))
            wkT = wk_p[hs_kv].reshape(NG * D, HID).T         # [HID, 256]
            wkT_t = np.ascontiguousarray(
                wkT.reshape(KT, P, NG, P).transpose(2, 1, 0, 3).astype(_BF))
            wvT = wv_r[hs_kv].reshape(NG * D, HID).T         # [HID, 256]
            wvT_t = np.ascontiguousarray(
                wvT.reshape(KT, P, NG * P).transpose(1, 0, 2).astype(_BF))
            woT = np.ascontiguousarray(
                wo.astype(f)[:, hg * NH * D:(hg + 1) * NH * D].T
                .reshape(NH, P, HID).transpose(1, 0, 2).astype(_BF))
            ckT = np.ascontiguousarray(
                cache_k[b].astype(f)[:, hs_kv][:, :, _PERM]
                .transpose(1, 2, 0).astype(_BF))
            cvP = np.ascontiguousarray(
                cache_v[b].astype(f)[:, hs_kv]
                .reshape(C // P, P, NG, P).transpose(1, 0, 2, 3).astype(_BF))
            in_maps.append({
                "hsT": hsT, "wqT": wqT_t, "wkT": wkT_t, "wvT": wvT_t,
                "woT": woT, "ckT": ckT, "cvP": cvP,
                "cs_cc": cs_cc, "cs_pm": cs_pm,
            })
    return in_maps


def gather_output(results):
    """Sum the 2 TP partials per batch -> full [B, S, HID] output."""
    out = np.empty((B, S, HID), np.float32)
    for b in range(B):
        out[b] = (results[2 * b]["y"].astype(np.float32)
                  + results[2 * b + 1]["y"].astype(np.float32))
    return out


def kernel(hidden_states, freqs_cos, freqs_sin, attention_mask,
           cache_k, cache_v, wq, wk, wv, wo):
    # attention_mask is all-zeros by construction (see spec) - unused.
    from concourse.bass_utils import run_bass_kernel_spmd

    nc = build_bass(unroll=1)
    in_maps = prep_inputs(
        np.asarray(hidden_states), np.asarray(freqs_cos), np.asarray(freqs_sin),
        np.asarray(cache_k), np.asarray(cache_v),
        np.asarray(wq), np.asarray(wk), np.asarray(wv), np.asarray(wo))
    res = run_bass_kernel_spmd(nc, in_maps, core_ids=list(range(N_CORES)))
    return gather_output(res.results)
